# revision 7
# baseline (speedup 1.0000x reference)
"""LDA reparameterized black-box VI ELBO on 8 Trainium2 NeuronCores.

Strategy (self-contained; shapes hardcoded from the problem spec):
  - Host: exact replication of the reference's Dirichlet sampling
    (jax CPU, key 42) -- data-dependent rejection sampling cannot be
    reproduced on-device. Also tiny O(D*K) doc-side terms and gammaln
    input constants.
  - Device (V-sharded across 8 cores, VC=6400 padded cols each):
      per sample s and 128-doc block: PE matmul doc_topics^T @ topics
      -> PSUM; ACT Ln(wp + 1e-10) -> bf16; DVE multiply by docs counts;
      PE ones-matmul reduction accumulated in PSUM -> log-lik partials.
      Plus ACT Ln(topics), DVE (lam-1)*ln(topics) + PE reduction for the
      topic entropy term, and DVE min-reduce of topics for exact
      zero-detection (log p(topics) = (eta-1)*ln(0) = nan semantics).
  - Host combine reproduces the reference's nan/inf propagation.
"""
import os
import numpy as np

V = 50257
K = 64
D = 512
S = 4
NCORES = 8
VC = 6400          # padded vocab columns per core (8*6400 = 51200 >= V)
VPAD = NCORES * VC
DB = 4             # d blocks of 128 docs
VTG = 5            # psum tiles of 1280 per (s, db); 5*1280 = 6400
TGW = 1280
ALPHA0 = 1.0
ETA0 = 1.0
EPS = 1e-10

_CACHE = {}


def _sample_dirichlet_cpu(topic_log_var, doc_log_var, n_samples):
    """Exactly replicate reference's sampling on the jax CPU backend."""
    os.environ.setdefault("JAX_PLATFORMS", "cpu")
    import jax

    try:
        jax.config.update("jax_platforms", "cpu")
    except Exception:
        pass
    import jax.numpy as jnp

    cpu = jax.devices("cpu")[0]
    with jax.default_device(cpu):
        lam = jnp.exp(jnp.asarray(topic_log_var))
        gam = jnp.exp(jnp.asarray(doc_log_var))
        kt, kd = jax.random.split(jax.random.key(42))
        topics = jax.random.dirichlet(kt, lam, shape=(n_samples, K))
        doc_topics = jax.random.dirichlet(kd, gam, shape=(n_samples, D))
        topics = np.asarray(jax.block_until_ready(topics))
        doc_topics = np.asarray(jax.block_until_ready(doc_topics))
    return topics, doc_topics


def _build_graph():
    import concourse.bacc as bacc
    import concourse.mybir as mybir
    import concourse.tile as tile

    F32 = mybir.dt.float32
    BF16 = mybir.dt.bfloat16
    AF = mybir.ActivationFunctionType
    ALU = mybir.AluOpType
    X = mybir.AxisListType.X

    nc = bacc.Bacc("TRN2", target_bir_lowering=False, debug=False,
                   num_devices=NCORES)
    docs_d = nc.dram_tensor("docs_bf", [128, DB * VC], BF16,
                            kind="ExternalInput").ap()
    tmm_d = nc.dram_tensor("topics_mm", [S, K, VC], BF16,
                           kind="ExternalInput").ap()
    tfl_d = nc.dram_tensor("topics_flat", [S, 128, K * VC // 128], F32,
                           kind="ExternalInput").ap()
    lam_d = nc.dram_tensor("lam_m1", [128, K * VC // 128], F32,
                           kind="ExternalInput").ap()
    dtT_d = nc.dram_tensor("dtT", [S, K, D], BF16, kind="ExternalInput").ap()
    outs_d = nc.dram_tensor("outs", [S, 2], F32, kind="ExternalOutput").ap()
    outB_d = nc.dram_tensor("outB", [128, S], F32, kind="ExternalOutput").ap()

    FL = K * VC // 128  # 3200

    with tile.TileContext(nc) as tc:
        with (
            tc.tile_pool(name="cst", bufs=1) as cst,
            tc.tile_pool(name="docsp", bufs=1) as docsp,
            tc.tile_pool(name="lamp", bufs=1) as lamp,
            tc.tile_pool(name="tmmp", bufs=2) as tmmp,
            tc.tile_pool(name="tflp", bufs=2) as tflp,
            tc.tile_pool(name="dtp", bufs=2) as dtp,
            tc.tile_pool(name="ltp", bufs=2) as ltp,
            tc.tile_pool(name="prap", bufs=2) as prap,
            tc.tile_pool(name="lnp", bufs=4) as lnp,
            tc.tile_pool(name="prp", bufs=4) as prp,
            tc.tile_pool(name="finp", bufs=1) as finp,
            tc.tile_pool(name="pmm", bufs=2, space="PSUM") as pmm,
            tc.tile_pool(name="pacc", bufs=1, space="PSUM") as pacc,
        ):
            eps_t = cst.tile([128, 1], F32, tag="eps")
            nc.gpsimd.memset(eps_t[:], EPS)
            # [128, 2S] one-hot window: col S-1... wait col S is ones; slice
            # [S-s : 2S-s] puts the ones-column at position s -> accumulates
            # sample s into psum row s (other rows += 0).
            ey_b = cst.tile([128, 2 * S], BF16, tag="ey_b")
            nc.gpsimd.memset(ey_b[:], 0.0)
            nc.gpsimd.memset(ey_b[:, S:S + 1], 1.0)
            ey_f = cst.tile([128, 2 * S], F32, tag="ey_f")
            nc.gpsimd.memset(ey_f[:], 0.0)
            nc.gpsimd.memset(ey_f[:, S:S + 1], 1.0)

            docs_t = docsp.tile([128, DB * VC], BF16, tag="docs")
            for db in range(DB):
                nc.sync.dma_start(docs_t[:, db * VC:(db + 1) * VC],
                                  docs_d[:, db * VC:(db + 1) * VC])
            lam_t = lamp.tile([128, FL], F32, tag="lam")
            nc.sync.dma_start(lam_t[:, :FL // 2], lam_d[:, :FL // 2])
            nc.sync.dma_start(lam_t[:, FL // 2:], lam_d[:, FL // 2:])

            # PSUM accumulators (1 bank each, partitions 0..S-1)
            LLp = pacc.tile([S, 512], F32, tag="LLp")
            Ap = pacc.tile([S, 512], F32, tag="Ap")
            Bacc = finp.tile([128, S], F32, tag="Bacc")
            fin = finp.tile([S, 2], F32, tag="fin")

            for s in range(S):
                tmm_s = tmmp.tile([K, VC], BF16, tag="tmm")
                nc.sync.dma_start(tmm_s[:], tmm_d[s])
                tfl_s = tflp.tile([128, FL], F32, tag="tfl")
                nc.sync.dma_start(tfl_s[:, :FL // 2], tfl_d[s][:, :FL // 2])
                nc.sync.dma_start(tfl_s[:, FL // 2:], tfl_d[s][:, FL // 2:])
                dt_s = dtp.tile([K, D], BF16, tag="dt")
                nc.sync.dma_start(dt_s[:], dtT_d[s])

                # topic-entropy stream: lt = ln(topics) (exact; Ln(0) = -inf)
                lt_s = ltp.tile([128, FL], F32, tag="lt")
                nc.scalar.activation(lt_s[:], tfl_s[:], AF.Ln, bias=0.0)
                prA = prap.tile([128, FL], F32, tag="prA")
                nc.vector.tensor_tensor(prA[:], lam_t[:], lt_s[:], op=ALU.mult)
                # reduce (partitions+chunks) on PE into Ap row s
                eyf_s = ey_f[:, S - s:2 * S - s]
                nA = FL // 512  # 6 full chunks of 512 + tail 128
                for c in range(nA):
                    nc.tensor.matmul(Ap[:, :512], eyf_s,
                                     prA[:, c * 512:(c + 1) * 512],
                                     start=(s == 0 and c == 0),
                                     stop=(s == S - 1 and c == nA - 1 and
                                           FL == nA * 512),
                                     skip_group_check=True)
                if FL > nA * 512:
                    nc.tensor.matmul(Ap[:, :FL - nA * 512], eyf_s,
                                     prA[:, nA * 512:], start=False,
                                     stop=(s == S - 1),
                                     skip_group_check=True)
                # zero-detect: min over topics values
                nc.vector.tensor_reduce(Bacc[:, s:s + 1], tfl_s[:], axis=X,
                                        op=ALU.min)

                # main log-lik stream
                eyb_s = ey_b[:, S - s:2 * S - s]
                for db in range(DB):
                    lhs = dt_s[:, db * 128:(db + 1) * 128]
                    for g in range(VTG):
                        base = g * TGW
                        ps_t = pmm.tile([128, TGW], F32, tag="pmm")
                        for (o, n) in ((0, 512), (512, 512), (1024, 256)):
                            nc.tensor.matmul(ps_t[:, o:o + n], lhs,
                                             tmm_s[:, base + o:base + o + n],
                                             start=True, stop=True)
                        ln_t = lnp.tile([128, TGW], BF16, tag="ln")
                        nc.scalar.activation(ln_t[:], ps_t[:], AF.Ln,
                                             bias=eps_t[:])
                        pr_t = prp.tile([128, TGW], BF16, tag="pr")
                        dslice = docs_t[:, db * VC + base:db * VC + base + TGW]
                        nc.vector.tensor_tensor(pr_t[:], ln_t[:], dslice,
                                                op=ALU.mult)
                        for (o, n) in ((0, 512), (512, 512), (1024, 256)):
                            first = (s == 0 and db == 0 and g == 0 and o == 0)
                            last = (s == S - 1 and db == DB - 1 and
                                    g == VTG - 1 and o == 1024)
                            nc.tensor.matmul(LLp[:, :n], eyb_s,
                                             pr_t[:, o:o + n],
                                             start=first, stop=last,
                                             skip_group_check=True)

            # fold [S,512] accumulators to scalars
            nc.vector.tensor_reduce(fin[:, 0:1], LLp[:], axis=X, op=ALU.add)
            nc.vector.tensor_reduce(fin[:, 1:2], Ap[:], axis=X, op=ALU.add)

            nc.sync.dma_start(outs_d[:], fin[:])
            nc.sync.dma_start(outB_d[:], Bacc[:])

    nc.compile()
    return nc


def _prep_core_inputs(topics, doc_topics, docs, lam):
    """Build the 8 per-core input dicts."""
    import ml_dtypes

    BF = ml_dtypes.bfloat16
    FL = K * VC // 128

    topics_pad = np.ones((S, K, VPAD), dtype=np.float32)
    topics_pad[:, :, :V] = topics
    lam_m1_pad = np.zeros((K, VPAD), dtype=np.float32)
    lam_m1_pad[:, :V] = lam - 1.0
    docs_pad = np.zeros((D, VPAD), dtype=np.float32)
    docs_pad[:, :V] = docs.astype(np.float32)

    dtT = np.ascontiguousarray(doc_topics.transpose(0, 2, 1)).astype(BF)

    in_maps = []
    for c in range(NCORES):
        sl = slice(c * VC, (c + 1) * VC)
        t_c = topics_pad[:, :, sl]
        in_maps.append({
            "docs_bf": np.ascontiguousarray(
                docs_pad[:, sl].reshape(DB, 128, VC).transpose(1, 0, 2)
            ).reshape(128, DB * VC).astype(BF),
            "topics_mm": np.ascontiguousarray(t_c).astype(BF),
            "topics_flat": np.ascontiguousarray(
                t_c.reshape(S, 128, FL)).astype(np.float32),
            "lam_m1": np.ascontiguousarray(
                lam_m1_pad[:, sl].reshape(128, FL)).astype(np.float32),
            "dtT": dtT,
        })
    return in_maps


def _register_ntff_hook():
    """Expose the axon NTFF profiling hook (missing antenv.axon_hooks)."""
    try:
        import sys
        import types

        if "antenv.axon_hooks" in sys.modules:
            return
        import trn_agent_boot.trn_boot as tb

        hook = tb._ntff_profile_via_ctypes("/opt/axon/libaxon_pjrt.so")
        mod = types.ModuleType("antenv.axon_hooks")
        mod._hook = hook
        mod.set_axon_ntff_profile_hook = lambda h: setattr(mod, "_hook", h)
        mod.get_axon_ntff_profile_hook = lambda: mod._hook
        sys.modules["antenv.axon_hooks"] = mod
        import antenv

        antenv.axon_hooks = mod
    except Exception as e:  # profiling is best-effort
        print("ntff hook registration failed:", e)


def kernel(topic_log_var, doc_log_var, docs, n_samples):
    from concourse.bass_utils import run_bass_kernel_spmd
    from scipy.special import gammaln

    topic_log_var = np.asarray(topic_log_var, dtype=np.float32)
    doc_log_var = np.asarray(doc_log_var, dtype=np.float32)
    docs = np.asarray(docs)
    n_samples = int(n_samples)
    assert n_samples == S and topic_log_var.shape == (K, V)

    topics, doc_topics = _sample_dirichlet_cpu(topic_log_var, doc_log_var,
                                               n_samples)
    lam = np.exp(topic_log_var)
    gam = np.exp(doc_log_var)

    if "nc" not in _CACHE:
        _CACHE["nc"] = _build_graph()
    nc = _CACHE["nc"]
    in_maps = _prep_core_inputs(topics, doc_topics, docs, lam)

    trace = os.environ.get("BASS_LDA_TRACE", "") == "1"
    if trace:
        _register_ntff_hook()
    res = run_bass_kernel_spmd(nc, in_maps, list(range(NCORES)), trace=trace,
                               tmpdir=os.environ.get("BASS_LDA_TRACEDIR"))
    if trace:
        print("HW exec time:", res.exec_time_ns, "ns")

    LL = np.zeros(S, dtype=np.float32)
    A = np.zeros(S, dtype=np.float32)
    Bmin = np.full(S, np.inf, dtype=np.float32)
    for c in range(NCORES):
        r = res.results[c]
        LL += r["outs"][:, 0]
        A += r["outs"][:, 1]
        Bmin = np.minimum(Bmin, r["outB"].min(axis=0))
    _CACHE["last"] = (LL.copy(), A.copy(), Bmin.copy(), topics, doc_topics)

    with np.errstate(all="ignore"):
        # input-derived gammaln constants (host; not sample-dependent)
        C_gam = np.float32(
            gammaln(lam.sum(axis=1, dtype=np.float64)).sum()
            - gammaln(lam.astype(np.float64)).sum())
        logp_t_const = np.float32(K * gammaln(float(V) * ETA0))
        logp_d_const = np.float32(D * gammaln(float(K) * ALPHA0))

        zeroflag = Bmin <= 0.0
        out = np.zeros(S, dtype=np.float32)
        for s in range(S):
            lt_any_zero = np.float32(np.nan) if zeroflag[s] else np.float32(0.0)
            logp_topics = logp_t_const + (ETA0 - 1.0) * 0.0 + lt_any_zero
            A_s = A[s] + C_gam
            if zeroflag[s]:
                A_s = np.float32(np.inf)

            ldt = np.log(doc_topics[s])  # (D,K), -inf at zeros
            logp_doc = np.float32(logp_d_const
                                  + ((ALPHA0 - 1.0) * ldt).sum(dtype=np.float32))
            lqd = np.float32(
                ((gam - 1.0) * ldt).sum(dtype=np.float32)
                + np.float32(gammaln(gam.sum(axis=1, dtype=np.float64)).sum()
                             - gammaln(gam.astype(np.float64)).sum()))
            out[s] = logp_topics + logp_doc + LL[s] - (A_s + lqd)
        elbo = np.float32(out.mean())
    return np.float32(-elbo)


if __name__ == "__main__":
    rng = np.random.default_rng(0)
    tlv = (rng.standard_normal((K, V)) * 0.01 + np.log(1.0 / V)).astype(np.float32)
    dlv = np.full((D, K), np.log(1.0 / K), dtype=np.float32)
    dcs = rng.integers(0, 5, (D, V)).astype(np.int32)
    print(kernel(tlv, dlv, dcs, S))
